# revision 2
# baseline (speedup 1.0000x reference)
"""BitFeedForward (BitNet-style FFN) Trainium2 kernel — 8-core data parallel.

kernel(**inputs) takes the FULL unsharded inputs of
nn_BitFeedForward_25280177504455:
    x  [4, 2048, 2048] f32, w1 [8192, 2048], b1 [8192],
    w2 [2048, 8192], b2 [2048]
and returns the full [4, 2048, 2048] f32 output.

Sharding: data-parallel over tokens (1024 tokens/core).  The host
precomputes the ternary weight form sign(w) (exact, a pure dtype/layout
transform of the input) plus the exact scalar mean|w| for each matrix,
and ships sign(w1) as fp8_e4m3 / sign(w2) as bf16 in slab-contiguous
layouts (16KB contiguous per partition line -> minimal DMA descriptor
count).  No on-device sign conversion and no sampled-mean pass.

Layer 1 runs the matmul in fp8 DoubleRow perf mode (contraction 256 per
instruction): activations are quantized per-token to int8 values and
then rounded to the fp8e4 grid (the only lossy step, ~1.8e-2 final max
rel err vs the fp32 reference, verified against the reference pipeline
in numpy).  Layer 2 activations stay exact bf16 (fp8 there would break
the 2e-2 budget).

Per-core flow:
  B. per token-tile: x stats -> per-tile scale finalize -> quantize
     (C_RND round trick) -> PE-transpose -> q1T (fp8) resident in SBUF.
  C. L1: stream w1 fp8 slabs, 8 DoubleRow matmuls per psum group,
     bias via vector ops on PSUM, gelu on scalar engine -> h bf16
     staged and spilled to DRAM in 2KB/partition chunks; per-token
     bn_stats/absmax accumulate.
  D. batched L2 scale finalize; q2 rebuilt from h (bf16) and
     PE-transposed into q2T (SBUF buffer aliased over q1T's).
  E. L2: stream w2 bf16 slab halves, 64 matmuls per psum group
     [out 128 x tok 512], beta2 scale via vector, bias b2 via scalar
     activation, out written bf16 [o, t]; host transposes and upcasts.
"""
import functools

import numpy as np
import ml_dtypes

from contextlib import ExitStack

import concourse.bacc as bacc
import concourse.tile as tile
from concourse import mybir
from concourse.bass_utils import run_bass_kernel_spmd

F32 = mybir.dt.float32
BF16 = mybir.dt.bfloat16
FP8 = mybir.dt.float8e4

EPS_RMS = 1e-6
EPS_Q = 1e-5
# v + C lands in [2^23, 2^24) where fp32 spacing is 1.0 -> RNE integer round
C_RND = float(1.5 * 2.0**23)
P = 128
AX = mybir.AxisListType
ALU = mybir.AluOpType
AF = mybir.ActivationFunctionType
DR = mybir.MatmulPerfMode.DoubleRow

NCORES = 8
B, S, DIM = 4, 2048, 2048
INNER = 8192
OUT = DIM
TOK = B * S // NCORES   # 1024 tokens per core
TT = TOK // P           # 8 token tiles
KD = DIM // P           # 16 contraction chunks for L1
KI = INNER // P         # 64 contraction chunks for L2
NE1 = 16                # L1 slabs (512 inner cols each)
SL1 = INNER // NE1      # 512
NB2 = 16                # L2 o-bands (128 out cols each)
BO = OUT // NB2         # 128


def build(fp8_l1=True, inter_b=True, weave=True):
    from concourse.tile_rust import add_dep_helper

    nc = bacc.Bacc("TRN2", enable_partition_id=False, num_devices=NCORES)

    WDT1 = FP8 if fp8_l1 else BF16
    x_d = nc.dram_tensor("x", [TOK, DIM], F32, kind="ExternalInput")
    # host slab layouts: w1s [e, p, dc, c] flat [NE1*P, KD*SL1],
    # w2s [b, p, kc, o] flat [NB2*P, KI*BO]; each slab row is contiguous.
    w1s_d = nc.dram_tensor("w1s", [NE1 * P, KD * SL1], WDT1,
                           kind="ExternalInput")
    w2s_d = nc.dram_tensor("w2s", [NB2 * P, KI * BO], BF16,
                           kind="ExternalInput")
    b1_d = nc.dram_tensor("b1", [1, INNER], BF16, kind="ExternalInput")
    b2c_d = nc.dram_tensor("b2c", [P, NB2], F32, kind="ExternalInput")
    mws_d = nc.dram_tensor("mws", [P, 2], F32, kind="ExternalInput")
    idf_d = nc.dram_tensor("identf", [P, P], F32, kind="ExternalInput")
    idb_d = nc.dram_tensor("identb", [P, P], BF16, kind="ExternalInput")
    out_d = nc.dram_tensor("out", [OUT, TOK], BF16, kind="ExternalOutput")

    with ExitStack() as ctx:
        tc = ctx.enter_context(tile.TileContext(nc))
        pool = lambda name, bufs, space="SBUF": ctx.enter_context(
            tc.tile_pool(name=name, bufs=bufs, space=space))

        consts = pool("consts", 1)
        xp = pool("xp", 2)            # f32 x half-tiles [P, 1024]
        q1p = pool("q1p", 2)          # bf16 q1 staging [P, 1024]
        wring = pool("wring", 4)      # weight slab halves (8KB/partition)
        qTp = pool("qTp", 1)          # q1T then q2T (aliased buffer)
        hcp = pool("hcp", 3)          # h bf16 spill tiles [P, 512]
        hrp = pool("hrp", 2)          # h bf16 reload [P, 1024]
        hqp = pool("hqp", 2)          # f32 q2 round staging [P, 512]
        q2cp = pool("q2cp", 2)        # bf16 q2 [P, 1024]
        btep = pool("btep", 2)        # f32 bias_te [P, 512]
        bbp = pool("bbp", 2)          # b1bc slab broadcast
        bch = pool("bch", 1)          # b1 row slices
        outp = pool("outp", 2)        # bf16 output drains
        vecs = pool("vecs", 2)
        pers = pool("pers", 1)
        dram = pool("dram", 1, "DRAM")
        ps_g = pool("ps_g", 4, "PSUM")
        ps_t = pool("ps_t", 2, "PSUM")
        ps_v = pool("ps_v", 2, "PSUM")

        identf = consts.tile([P, P], F32)
        identb = consts.tile([P, P], BF16)
        b2c = consts.tile([P, NB2], F32)
        mws = consts.tile([P, 2], F32)
        nc.sync.dma_start(identf, idf_d[:, :])
        nc.sync.dma_start(identb, idb_d[:, :])
        nc.sync.dma_start(b2c, b2c_d[:, :])
        nc.sync.dma_start(mws, mws_d[:, :])

        h_dram = dram.tile([TT, P, INNER], BF16)

        state = {"pe": None}

        def pe(instr):
            if state["pe"] is not None:
                add_dep_helper(instr.ins, state["pe"].ins, sync=False,
                               reason="pe chain")
            state["pe"] = instr
            return instr

        # ---- persistent scalars/vectors ----
        stvx = pers.tile([P, TT, 4, 6], F32, tag="stvx")
        M1s = pers.tile([P, TT], F32, tag="M1s")
        r1s = pers.tile([P, TT], F32, tag="r1s")
        c1s = pers.tile([P, TT], F32, tag="c1s")
        beta1s = pers.tile([P, TT], F32, tag="beta1s")
        rb1s = pers.tile([P, TT], F32, tag="rb1s")
        r2s = pers.tile([P, TT], F32, tag="r2s")
        m2s = pers.tile([P, TT], F32, tag="m2s")
        c2s = pers.tile([P, TT], F32, tag="c2s")
        beta2s = pers.tile([P, TT], F32, tag="beta2s")
        beta2row = pers.tile([1, TOK], F32, tag="beta2row")
        stv2 = pers.tile([P, TT, NE1, 6], F32, tag="stv2")
        bb0 = pers.tile([P, 512], F32, tag="bb0")
        bb1 = pers.tile([P, 512], F32, tag="bb1")

        def finalize(stv_all, M_all, WID, r_all, c_all, sl):
            # sl: token-tile slice (per-tt [tt:tt+1] or all [0:TT])
            nt = sl.stop - sl.start
            mvs = vecs.tile([P, TT, 2], F32, tag="bn_mvs")
            for tt in range(sl.start, sl.stop):
                nc.vector.bn_aggr(mvs[:, tt, :], stv_all[:, tt, :, :])
            msq = vecs.tile([P, TT], F32, tag="msqs")
            m_ = mvs[:, sl, 0]
            v_ = mvs[:, sl, 1]
            q_ = msq[:, sl]
            nc.vector.tensor_tensor(q_, m_, m_, op=ALU.mult)
            nc.vector.tensor_tensor(q_, q_, v_, op=ALU.add)
            nc.vector.tensor_scalar_add(q_, q_, EPS_RMS)
            y = vecs.tile([P, TT], F32, tag="sq_ys")
            nc.scalar.sqrt(y[:, sl], q_)
            d_ = vecs.tile([P, TT], F32, tag="sq_ds")
            nc.vector.reciprocal(d_[:, sl], y[:, sl])
            nc.vector.tensor_tensor(d_[:, sl], q_, d_[:, sl], op=ALU.mult)
            nc.vector.tensor_tensor(y[:, sl], y[:, sl], d_[:, sl],
                                    op=ALU.add)
            nc.vector.tensor_scalar_mul(y[:, sl], y[:, sl],
                                        0.5 * (float(WID) ** 0.5))
            a = vecs.tile([P, TT], F32, tag="as_")
            nc.vector.reciprocal(a[:, sl], y[:, sl])
            nc.vector.tensor_tensor(c_all[:, sl], a[:, sl], M_all[:, sl],
                                    op=ALU.mult)
            nc.vector.tensor_scalar_max(c_all[:, sl], c_all[:, sl], EPS_Q)
            r = vecs.tile([P, TT], F32, tag="rs_")
            nc.vector.reciprocal(r[:, sl], c_all[:, sl])
            nc.vector.tensor_tensor(r[:, sl], r[:, sl], a[:, sl],
                                    op=ALU.mult)
            nc.vector.tensor_scalar_mul(r_all[:, sl], r[:, sl], 127.0)

        def col_to_row(col, row_slice):
            pst = ps_v.tile([1, P], F32, tag="psv")
            pe(nc.tensor.transpose(pst, col, identf))
            nc.scalar.copy(row_slice, pst)

        def load_slab1(e):
            wh = [wring.tile([P, KD // 2, SL1], WDT1, tag="w",
                             name=f"ws1_{e}_{h}") for h in range(2)]
            for h in range(2):
                nc.sync.dma_start(
                    wh[h], w1s_d[e * P:(e + 1) * P,
                                 h * (KD // 2) * SL1:
                                 (h + 1) * (KD // 2) * SL1].rearrange(
                        "p (k c) -> p k c", c=SL1))
            bc = bch.tile([1, SL1], BF16, tag="bc")
            nc.sync.dma_start(bc, b1_d[0:1, e * SL1:(e + 1) * SL1])
            b1bc = bbp.tile([P, SL1], BF16, tag="b1bc", name=f"b1bc_{e}")
            nc.gpsimd.partition_broadcast(b1bc, bc)
            return wh, b1bc

        def l1_group(e, tt, wh, b1bc):
            pg = ps_g.tile([P, SL1], F32, tag="psg")
            ts_ = slice(tt * P, (tt + 1) * P)
            if fp8_l1:
                for dcp in range(KD // 2):
                    h, k = divmod(dcp, 4)
                    pe(nc.tensor.matmul(
                        pg, q1T[:, 2 * dcp:2 * dcp + 2, ts_],
                        wh[h][:, 2 * k:2 * k + 2, :],
                        start=(dcp == 0), stop=(dcp == KD // 2 - 1),
                        perf_mode=DR))
            else:
                for dc in range(KD):
                    h, k = divmod(dc, KD // 2)
                    pe(nc.tensor.matmul(
                        pg, q1T[:, dc, ts_], wh[h][:, k, :],
                        start=(dc == 0), stop=(dc == KD - 1)))
            bias_te = btep.tile([P, SL1], BF16, tag="bte")
            nc.vector.tensor_scalar(bias_te, b1bc, rb1s[:, tt:tt + 1],
                                    None, op0=ALU.mult)
            nc.vector.tensor_tensor(pg, pg, bias_te, op=ALU.add)
            hc = hcp.tile([P, SL1], BF16, tag="hc")
            nc.scalar.activation(hc, pg, AF.Gelu,
                                 scale=beta1s[:, tt:tt + 1])
            nc.vector.bn_stats(stv2[:, tt, e, :], hc)
            mx = vecs.tile([P, 1], F32, tag="mx")
            nc.vector.tensor_reduce(mx, hc, axis=AX.X, op=ALU.max,
                                    apply_absolute_value=True)
            m2 = m2s[:, tt:tt + 1]
            if e == 0:
                nc.vector.tensor_copy(out=m2, in_=mx)
            else:
                nc.vector.tensor_tensor(m2, m2, mx, op=ALU.max)
            nc.sync.dma_start(h_dram[tt, :, e * 512:(e + 1) * 512], hc)

        # ===== B: per-tile x stats + finalize + quantize + transpose ===
        # L1 slab 0 (and 1) stream in during the stats pass; the first
        # matmul group for token tile tt is emitted right behind tt's
        # transposes so the PE queue never drains behind phase B.
        q1T = qTp.tile([P, KD, TOK], WDT1, tag="qT", name="q1T")
        slabs = {0: load_slab1(0), 1: load_slab1(1)}
        for tt in range(TT):
            xh = []
            for h in range(2):
                xt = xp.tile([P, 1024], F32, tag="x", name=f"x_{tt}_{h}")
                nc.sync.dma_start(
                    xt, x_d[tt * P:(tt + 1) * P, h * 1024:(h + 1) * 1024])
                xh.append(xt)
                for c in range(2):
                    nc.vector.bn_stats(stvx[:, tt, 2 * h + c, :],
                                       xt[:, c * 512:(c + 1) * 512])
                mx = vecs.tile([P, 1], F32, tag="mx")
                nc.vector.tensor_reduce(mx, xt, axis=AX.X, op=ALU.max,
                                        apply_absolute_value=True)
                M1 = M1s[:, tt:tt + 1]
                if h == 0:
                    nc.vector.tensor_copy(out=M1, in_=mx)
                else:
                    nc.vector.tensor_tensor(M1, M1, mx, op=ALU.max)
            finalize(stvx, M1s, DIM, r1s, c1s, slice(tt, tt + 1))
            nc.vector.tensor_scalar(beta1s[:, tt:tt + 1], c1s[:, tt:tt + 1],
                                    mws[:, 0:1], None, op0=ALU.mult)
            nc.vector.reciprocal(rb1s[:, tt:tt + 1], beta1s[:, tt:tt + 1])
            # v = x*r1 + C_RND ; q1 = v - C_RND (RNE integer round)
            for h in range(2):
                nc.vector.tensor_scalar(xh[h], xh[h], r1s[:, tt:tt + 1],
                                        C_RND, op0=ALU.mult, op1=ALU.add)
                q1 = q1p.tile([P, 1024], BF16, tag="q1")
                nc.scalar.activation(q1, xh[h], AF.Copy, bias=-C_RND)
                pst = ps_t.tile([P, 1024], BF16, tag="pst")
                for j in range(8):
                    pe(nc.tensor.transpose(
                        pst[:, j * P:(j + 1) * P],
                        q1[:, j * P:(j + 1) * P], identb))
                nc.vector.tensor_copy(
                    out=q1T[:, 8 * h:8 * (h + 1), tt * P:(tt + 1) * P],
                    in_=pst.rearrange("p (a b) -> p a b", b=P))
            if inter_b:
                l1_group(0, tt, *slabs[0])
        if not inter_b:
            for tt in range(TT):
                l1_group(0, tt, *slabs[0])

        # ===== C: remaining L1 slabs ===================================
        # In the last slab, the per-token L2 scale finalize is emitted
        # right behind each group so it overlaps the L1 tail.
        for e in range(1, NE1):
            if e + 1 < NE1:
                slabs[e + 1] = load_slab1(e + 1)
            for tt in range(TT):
                l1_group(e, tt, *slabs[e])
                if e == NE1 - 1:
                    finalize(stv2, m2s, INNER, r2s, c2s, slice(tt, tt + 1))
                    nc.vector.tensor_scalar(beta2s[:, tt:tt + 1],
                                            c2s[:, tt:tt + 1],
                                            mws[:, 1:2], None, op0=ALU.mult)
                    col_to_row(beta2s[:, tt:tt + 1],
                               beta2row[0:1, tt * P:(tt + 1) * P])
            del slabs[e - 1]

        nc.gpsimd.partition_broadcast(bb0, beta2row[0:1, 0:512])
        nc.gpsimd.partition_broadcast(bb1, beta2row[0:1, 512:1024])
        bbs = [bb0, bb1]

        q2T = qTp.tile([P, KI, TOK], BF16, tag="qT", name="q2T")

        def q2_build(tt):
            for icp in range(INNER // 1024):
                hr = hrp.tile([P, 1024], BF16, tag="hr")
                nc.sync.dma_start(
                    hr, h_dram[tt, :, icp * 1024:(icp + 1) * 1024])
                q2c = q2cp.tile([P, 1024], BF16, tag="q2c")
                for hh in range(2):
                    hq = hqp.tile([P, 512], F32, tag="hq")
                    nc.scalar.activation(hq, hr[:, hh * 512:(hh + 1) * 512],
                                         AF.Copy, bias=C_RND,
                                         scale=r2s[:, tt:tt + 1])
                    nc.vector.tensor_scalar(q2c[:, hh * 512:(hh + 1) * 512],
                                            hq, C_RND, None,
                                            op0=ALU.subtract)
                pst = ps_t.tile([P, 1024], BF16, tag="pst")
                for j in range(8):
                    pe(nc.tensor.transpose(
                        pst[:, j * P:(j + 1) * P],
                        q2c[:, j * P:(j + 1) * P], identb))
                nc.vector.tensor_copy(
                    out=q2T[:, 8 * icp:8 * (icp + 1), tt * P:(tt + 1) * P],
                    in_=pst.rearrange("p (a b) -> p a b", b=P))

        # ===== E: L2 o-bands, token-half sweeps ========================
        # tg0's bands run while tg1's q2T is still being rebuilt (the
        # build transposes slot between band matmul groups on the PE
        # queue); w2 slabs are streamed once per sweep.
        def band(b, s, tg):
            w2h = [wring.tile([P, KI // 2, BO], BF16, tag="w",
                              name=f"ws2_{s}_{b}_{h}") for h in range(2)]
            for h in range(2):
                nc.sync.dma_start(
                    w2h[h], w2s_d[b * P:(b + 1) * P,
                                  h * (KI // 2) * BO:
                                  (h + 1) * (KI // 2) * BO].rearrange(
                        "p (k o) -> p k o", o=BO))
            pg2 = ps_g.tile([P, 512], F32, tag="psg", name=f"pb{s}_{b}")
            tsl = slice(tg * 512, (tg + 1) * 512)
            for kc in range(KI):
                h, k = divmod(kc, KI // 2)
                pe(nc.tensor.matmul(pg2, w2h[h][:, k, :],
                                    q2T[:, kc, tsl],
                                    start=(kc == 0),
                                    stop=(kc == KI - 1)))
            nc.vector.tensor_tensor(pg2, pg2, bbs[tg], op=ALU.mult)
            ob = outp.tile([P, 512], BF16, tag="ob")
            nc.vector.tensor_scalar(ob, pg2, b2c[:, b:b + 1], None,
                                    op0=ALU.add)
            nc.sync.dma_start(out_d[b * BO:(b + 1) * BO, tsl], ob)

        def band2(b):
            w2h = [wring.tile([P, KI // 2, BO], BF16, tag="w",
                              name=f"w2s_{b}_{h}") for h in range(2)]
            for h in range(2):
                nc.sync.dma_start(
                    w2h[h], w2s_d[b * P:(b + 1) * P,
                                  h * (KI // 2) * BO:
                                  (h + 1) * (KI // 2) * BO].rearrange(
                        "p (k o) -> p k o", o=BO))
            for tg in range(2):
                pg2 = ps_g.tile([P, 512], F32, tag="psg",
                                name=f"pb2_{b}_{tg}")
                tsl = slice(tg * 512, (tg + 1) * 512)
                for kc in range(KI):
                    h, k = divmod(kc, KI // 2)
                    pe(nc.tensor.matmul(pg2, w2h[h][:, k, :],
                                        q2T[:, kc, tsl],
                                        start=(kc == 0),
                                        stop=(kc == KI - 1)))
                nc.vector.tensor_tensor(pg2, pg2, bbs[tg], op=ALU.mult)
                ob = outp.tile([P, 512], BF16, tag="ob")
                nc.vector.tensor_scalar(ob, pg2, b2c[:, b:b + 1], None,
                                        op0=ALU.add)
                nc.sync.dma_start(out_d[b * BO:(b + 1) * BO, tsl], ob)

        if weave:
            for tt in range(4):
                q2_build(tt)
            for b in range(NB2):
                band(b, 0, 0)
                if b < 4:
                    q2_build(4 + b)
            for b in range(NB2):
                band(b, 1, 1)
        else:
            for tt in range(TT):
                q2_build(tt)
            for b in range(NB2):
                band2(b)

    nc.compile()
    return nc


@functools.lru_cache(maxsize=4)
def _get_nc(fp8_l1=True):
    import os
    inter_b = os.environ.get("K_INTER_B", "0") == "1"
    weave = os.environ.get("K_WEAVE", "0") == "1"
    return build(fp8_l1, inter_b, weave)


def _prep_weights(w1, w2, fp8_l1):
    w1 = np.asarray(w1, dtype=np.float32)
    w2 = np.asarray(w2, dtype=np.float32)
    mw1 = float(np.mean(np.abs(w1), dtype=np.float64))
    mw2 = float(np.mean(np.abs(w2), dtype=np.float64))
    wdt1 = ml_dtypes.float8_e4m3 if fp8_l1 else ml_dtypes.bfloat16
    s1t = np.sign(w1).T.astype(wdt1)        # [DIM, INNER]
    s2t = np.sign(w2).T.astype(ml_dtypes.bfloat16)  # [INNER, OUT]
    # w1s[e, p, dc, c] = s1t[dc*128+p, e*512+c]
    w1s = np.ascontiguousarray(
        s1t.reshape(KD, P, NE1, SL1).transpose(2, 1, 0, 3)
        .reshape(NE1 * P, KD * SL1))
    # w2s[b, p, kc, o] = s2t[kc*128+p, b*128+o]
    w2s = np.ascontiguousarray(
        s2t.reshape(KI, P, NB2, BO).transpose(2, 1, 0, 3)
        .reshape(NB2 * P, KI * BO))
    return w1s, w2s, mw1, mw2


def kernel(x, w1, b1, w2, b2, _trace=False, _fp8=True):
    nc = _get_nc(_fp8)
    xf = np.ascontiguousarray(x.reshape(B * S, DIM), dtype=np.float32)
    w1s, w2s, mw1, mw2 = _prep_weights(w1, w2, _fp8)
    b2f = np.asarray(b2, dtype=np.float32)
    common = {
        "w1s": w1s,
        "w2s": w2s,
        "b1": np.asarray(b1, dtype=np.float32).reshape(1, INNER).astype(
            ml_dtypes.bfloat16),
        "b2c": np.ascontiguousarray(
            b2f.reshape(NB2, P).T.astype(np.float32)),
        "mws": np.tile(np.array([[mw1 / 127.0, mw2 / 127.0]],
                                dtype=np.float32), (P, 1)),
        "identf": np.eye(P, dtype=np.float32),
        "identb": np.eye(P, dtype=np.float32).astype(ml_dtypes.bfloat16),
    }
    in_maps = []
    for c in range(NCORES):
        in_maps.append({
            "x": xf[c * TOK:(c + 1) * TOK],
            **common,
        })
    res = run_bass_kernel_spmd(nc, in_maps, core_ids=list(range(NCORES)),
                               trace=_trace)
    out = np.concatenate(
        [res.results[c]["out"].astype(np.float32).T for c in range(NCORES)],
        axis=0)
    out = out.reshape(B, S, DIM)
    if _trace:
        return out, res
    return out


# revision 3
# speedup vs baseline: 1.0704x; 1.0704x over previous
"""BitFeedForward (BitNet-style FFN) Trainium2 kernel — 8-core data parallel.

kernel(**inputs) takes the FULL unsharded inputs of
nn_BitFeedForward_25280177504455:
    x  [4, 2048, 2048] f32, w1 [8192, 2048], b1 [8192],
    w2 [2048, 8192], b2 [2048]
and returns the full [4, 2048, 2048] f32 output.

Sharding: data-parallel over tokens (1024 tokens/core).  The host
precomputes the ternary weight form sign(w) (exact, a pure dtype/layout
transform of the input) plus the exact scalar mean|w| for each matrix,
and ships sign(w1) as fp8_e4m3 / sign(w2) as bf16 in slab-contiguous
layouts (16KB contiguous per partition line -> minimal DMA descriptor
count).  No on-device sign conversion and no sampled-mean pass.

Layer 1 runs the matmul in fp8 DoubleRow perf mode (contraction 256 per
instruction): activations are quantized per-token to int8 values and
then rounded to the fp8e4 grid (the only lossy step, ~1.8e-2 final max
rel err vs the fp32 reference, verified against the reference pipeline
in numpy).  Layer 2 activations stay exact bf16 (fp8 there would break
the 2e-2 budget).

Per-core flow:
  B. per token-tile: x stats -> per-tile scale finalize -> quantize
     (C_RND round trick) -> PE-transpose -> q1T (fp8) resident in SBUF.
  C. L1: stream w1 fp8 slabs, 8 DoubleRow matmuls per psum group,
     bias via vector ops on PSUM, gelu on scalar engine -> h bf16
     staged and spilled to DRAM in 2KB/partition chunks; per-token
     bn_stats/absmax accumulate.
  D. batched L2 scale finalize; q2 rebuilt from h (bf16) and
     PE-transposed into q2T (SBUF buffer aliased over q1T's).
  E. L2: stream w2 bf16 slab halves, 64 matmuls per psum group
     [out 128 x tok 512], beta2 scale via vector, bias b2 via scalar
     activation, out written bf16 [o, t]; host transposes and upcasts.
"""
import functools

import numpy as np
import ml_dtypes

from contextlib import ExitStack

import concourse.bacc as bacc
import concourse.tile as tile
from concourse import mybir
from concourse.bass_utils import run_bass_kernel_spmd

F32 = mybir.dt.float32
BF16 = mybir.dt.bfloat16
FP8 = mybir.dt.float8e4

EPS_RMS = 1e-6
EPS_Q = 1e-5
# v + C lands in [2^23, 2^24) where fp32 spacing is 1.0 -> RNE integer round
C_RND = float(1.5 * 2.0**23)
P = 128
AX = mybir.AxisListType
ALU = mybir.AluOpType
AF = mybir.ActivationFunctionType
DR = mybir.MatmulPerfMode.DoubleRow

NCORES = 8
B, S, DIM = 4, 2048, 2048
INNER = 8192
OUT = DIM
TOK = B * S // NCORES   # 1024 tokens per core
TT = TOK // P           # 8 token tiles
KD = DIM // P           # 16 contraction chunks for L1
KI = INNER // P         # 64 contraction chunks for L2
NE1 = 16                # L1 slabs (512 inner cols each)
SL1 = INNER // NE1      # 512
NB2 = 16                # L2 o-bands (128 out cols each)
BO = OUT // NB2         # 128


def build(fp8_l1=True, inter_b=True, weave=True):
    from concourse.tile_rust import add_dep_helper

    nc = bacc.Bacc("TRN2", enable_partition_id=False, num_devices=NCORES)

    WDT1 = FP8 if fp8_l1 else BF16
    x_d = nc.dram_tensor("x", [TOK, DIM], F32, kind="ExternalInput")
    # host slab layouts: w1s [e, p, dc, c] flat [NE1*P, KD*SL1],
    # w2s [b, p, kc, o] flat [NB2*P, KI*BO]; each slab row is contiguous.
    w1s_d = nc.dram_tensor("w1s", [NE1 * P, KD * SL1], WDT1,
                           kind="ExternalInput")
    w2s_d = nc.dram_tensor("w2s", [NB2 * P, KI * BO], BF16,
                           kind="ExternalInput")
    b1_d = nc.dram_tensor("b1", [1, INNER], BF16, kind="ExternalInput")
    b2c_d = nc.dram_tensor("b2c", [P, NB2], F32, kind="ExternalInput")
    mws_d = nc.dram_tensor("mws", [P, 2], F32, kind="ExternalInput")
    idf_d = nc.dram_tensor("identf", [P, P], F32, kind="ExternalInput")
    idb_d = nc.dram_tensor("identb", [P, P], BF16, kind="ExternalInput")
    out_d = nc.dram_tensor("out", [OUT, TOK], BF16, kind="ExternalOutput")

    with ExitStack() as ctx:
        tc = ctx.enter_context(tile.TileContext(nc))
        pool = lambda name, bufs, space="SBUF": ctx.enter_context(
            tc.tile_pool(name=name, bufs=bufs, space=space))

        consts = pool("consts", 1)
        xp = pool("xp", 2)            # f32 x half-tiles [P, 1024]
        q1p = pool("q1p", 2)          # bf16 q1 staging [P, 1024]
        wring = pool("wring", 4)      # weight slab halves (8KB/partition)
        qTp = pool("qTp", 1)          # q1T then q2T (aliased buffer)
        hcp = pool("hcp", 3)          # h bf16 spill tiles [P, 512]
        hrp = pool("hrp", 2)          # h bf16 reload [P, 1024]
        hqp = pool("hqp", 2)          # f32 q2 round staging [P, 512]
        q2cp = pool("q2cp", 2)        # bf16 q2 [P, 1024]
        btep = pool("btep", 2)        # f32 bias_te [P, 512]
        bbp = pool("bbp", 2)          # b1bc slab broadcast
        bch = pool("bch", 1)          # b1 row slices
        outp = pool("outp", 2)        # bf16 output drains
        vecs = pool("vecs", 2)
        pers = pool("pers", 1)
        dram = pool("dram", 1, "DRAM")
        ps_g = pool("ps_g", 4, "PSUM")
        ps_t = pool("ps_t", 2, "PSUM")
        ps_v = pool("ps_v", 2, "PSUM")

        identf = consts.tile([P, P], F32)
        identb = consts.tile([P, P], BF16)
        b2c = consts.tile([P, NB2], F32)
        mws = consts.tile([P, 2], F32)
        nc.sync.dma_start(identf, idf_d[:, :])
        nc.sync.dma_start(identb, idb_d[:, :])
        nc.sync.dma_start(b2c, b2c_d[:, :])
        nc.sync.dma_start(mws, mws_d[:, :])

        h_dram = dram.tile([TT, P, INNER], BF16)

        state = {"pe": None}

        def pe(instr):
            if state["pe"] is not None:
                add_dep_helper(instr.ins, state["pe"].ins, sync=False,
                               reason="pe chain")
            state["pe"] = instr
            return instr

        # ---- persistent scalars/vectors ----
        stvx = pers.tile([P, TT, 4, 6], F32, tag="stvx")
        M1s = pers.tile([P, TT], F32, tag="M1s")
        r1s = pers.tile([P, TT], F32, tag="r1s")
        c1s = pers.tile([P, TT], F32, tag="c1s")
        beta1s = pers.tile([P, TT], F32, tag="beta1s")
        rb1s = pers.tile([P, TT], F32, tag="rb1s")
        r2s = pers.tile([P, TT], F32, tag="r2s")
        m2s = pers.tile([P, TT], F32, tag="m2s")
        c2s = pers.tile([P, TT], F32, tag="c2s")
        beta2s = pers.tile([P, TT], F32, tag="beta2s")
        beta2row = pers.tile([1, TOK], F32, tag="beta2row")
        stv2 = pers.tile([P, TT, NE1, 6], F32, tag="stv2")
        bb0 = pers.tile([P, 512], F32, tag="bb0")
        bb1 = pers.tile([P, 512], F32, tag="bb1")

        def finalize(stv_all, M_all, WID, r_all, c_all, sl):
            # sl: token-tile slice (per-tt [tt:tt+1] or all [0:TT])
            nt = sl.stop - sl.start
            mvs = vecs.tile([P, TT, 2], F32, tag="bn_mvs")
            for tt in range(sl.start, sl.stop):
                nc.vector.bn_aggr(mvs[:, tt, :], stv_all[:, tt, :, :])
            msq = vecs.tile([P, TT], F32, tag="msqs")
            m_ = mvs[:, sl, 0]
            v_ = mvs[:, sl, 1]
            q_ = msq[:, sl]
            nc.vector.tensor_tensor(q_, m_, m_, op=ALU.mult)
            nc.vector.tensor_tensor(q_, q_, v_, op=ALU.add)
            nc.vector.tensor_scalar_add(q_, q_, EPS_RMS)
            y = vecs.tile([P, TT], F32, tag="sq_ys")
            nc.scalar.sqrt(y[:, sl], q_)
            d_ = vecs.tile([P, TT], F32, tag="sq_ds")
            nc.vector.reciprocal(d_[:, sl], y[:, sl])
            nc.vector.tensor_tensor(d_[:, sl], q_, d_[:, sl], op=ALU.mult)
            nc.vector.tensor_tensor(y[:, sl], y[:, sl], d_[:, sl],
                                    op=ALU.add)
            nc.vector.tensor_scalar_mul(y[:, sl], y[:, sl],
                                        0.5 * (float(WID) ** 0.5))
            a = vecs.tile([P, TT], F32, tag="as_")
            nc.vector.reciprocal(a[:, sl], y[:, sl])
            nc.vector.tensor_tensor(c_all[:, sl], a[:, sl], M_all[:, sl],
                                    op=ALU.mult)
            nc.vector.tensor_scalar_max(c_all[:, sl], c_all[:, sl], EPS_Q)
            r = vecs.tile([P, TT], F32, tag="rs_")
            nc.vector.reciprocal(r[:, sl], c_all[:, sl])
            nc.vector.tensor_tensor(r[:, sl], r[:, sl], a[:, sl],
                                    op=ALU.mult)
            nc.vector.tensor_scalar_mul(r_all[:, sl], r[:, sl], 127.0)

        def col_to_row(col, row_slice):
            pst = ps_v.tile([1, P], F32, tag="psv")
            pe(nc.tensor.transpose(pst, col, identf))
            nc.scalar.copy(row_slice, pst)

        def load_slab1(e):
            wh = [wring.tile([P, KD // 2, SL1], WDT1, tag="w",
                             name=f"ws1_{e}_{h}") for h in range(2)]
            for h in range(2):
                nc.sync.dma_start(
                    wh[h], w1s_d[e * P:(e + 1) * P,
                                 h * (KD // 2) * SL1:
                                 (h + 1) * (KD // 2) * SL1].rearrange(
                        "p (k c) -> p k c", c=SL1))
            bc = bch.tile([1, SL1], BF16, tag="bc")
            nc.sync.dma_start(bc, b1_d[0:1, e * SL1:(e + 1) * SL1])
            b1bc = bbp.tile([P, SL1], BF16, tag="b1bc", name=f"b1bc_{e}")
            nc.gpsimd.partition_broadcast(b1bc, bc)
            return wh, b1bc

        def l1_group(e, tt, wh, b1bc):
            pg = ps_g.tile([P, SL1], F32, tag="psg")
            ts_ = slice(tt * P, (tt + 1) * P)
            if fp8_l1:
                for dcp in range(KD // 2):
                    h, k = divmod(dcp, 4)
                    pe(nc.tensor.matmul(
                        pg, q1T[:, 2 * dcp:2 * dcp + 2, ts_],
                        wh[h][:, 2 * k:2 * k + 2, :],
                        start=(dcp == 0), stop=(dcp == KD // 2 - 1),
                        perf_mode=DR))
            else:
                for dc in range(KD):
                    h, k = divmod(dc, KD // 2)
                    pe(nc.tensor.matmul(
                        pg, q1T[:, dc, ts_], wh[h][:, k, :],
                        start=(dc == 0), stop=(dc == KD - 1)))
            bias_te = btep.tile([P, SL1], BF16, tag="bte")
            nc.vector.tensor_scalar(bias_te, b1bc, rb1s[:, tt:tt + 1],
                                    None, op0=ALU.mult)
            nc.vector.tensor_tensor(pg, pg, bias_te, op=ALU.add)
            hc = hcp.tile([P, SL1], BF16, tag="hc")
            nc.scalar.activation(hc, pg, AF.Gelu,
                                 scale=beta1s[:, tt:tt + 1])
            nc.vector.bn_stats(stv2[:, tt, e, :], hc)
            mx = vecs.tile([P, 1], F32, tag="mx")
            nc.vector.tensor_reduce(mx, hc, axis=AX.X, op=ALU.max,
                                    apply_absolute_value=True)
            m2 = m2s[:, tt:tt + 1]
            if e == 0:
                nc.vector.tensor_copy(out=m2, in_=mx)
            else:
                nc.vector.tensor_tensor(m2, m2, mx, op=ALU.max)
            nc.sync.dma_start(h_dram[tt, :, e * 512:(e + 1) * 512], hc)

        # ===== B: per-tile x stats + finalize + quantize + transpose ===
        # L1 slab 0 (and 1) stream in during the stats pass; the first
        # matmul group for token tile tt is emitted right behind tt's
        # transposes so the PE queue never drains behind phase B.
        q1T = qTp.tile([P, KD, TOK], WDT1, tag="qT", name="q1T")
        slabs = {0: load_slab1(0), 1: load_slab1(1)}
        for tt in range(TT):
            xh = []
            for h in range(2):
                xt = xp.tile([P, 1024], F32, tag="x", name=f"x_{tt}_{h}")
                nc.sync.dma_start(
                    xt, x_d[tt * P:(tt + 1) * P, h * 1024:(h + 1) * 1024])
                xh.append(xt)
                for c in range(2):
                    nc.vector.bn_stats(stvx[:, tt, 2 * h + c, :],
                                       xt[:, c * 512:(c + 1) * 512])
                mx = vecs.tile([P, 1], F32, tag="mx")
                nc.vector.tensor_reduce(mx, xt, axis=AX.X, op=ALU.max,
                                        apply_absolute_value=True)
                M1 = M1s[:, tt:tt + 1]
                if h == 0:
                    nc.vector.tensor_copy(out=M1, in_=mx)
                else:
                    nc.vector.tensor_tensor(M1, M1, mx, op=ALU.max)
            finalize(stvx, M1s, DIM, r1s, c1s, slice(tt, tt + 1))
            nc.vector.tensor_scalar(beta1s[:, tt:tt + 1], c1s[:, tt:tt + 1],
                                    mws[:, 0:1], None, op0=ALU.mult)
            nc.vector.reciprocal(rb1s[:, tt:tt + 1], beta1s[:, tt:tt + 1])
            # v = x*r1 + C_RND ; q1 = v - C_RND (RNE integer round)
            for h in range(2):
                nc.vector.tensor_scalar(xh[h], xh[h], r1s[:, tt:tt + 1],
                                        C_RND, op0=ALU.mult, op1=ALU.add)
                q1 = q1p.tile([P, 1024], BF16, tag="q1")
                nc.scalar.activation(q1, xh[h], AF.Copy, bias=-C_RND)
                pst = ps_t.tile([P, 1024], BF16, tag="pst")
                for j in range(8):
                    pe(nc.tensor.transpose(
                        pst[:, j * P:(j + 1) * P],
                        q1[:, j * P:(j + 1) * P], identb))
                nc.vector.tensor_copy(
                    out=q1T[:, 8 * h:8 * (h + 1), tt * P:(tt + 1) * P],
                    in_=pst.rearrange("p (a b) -> p a b", b=P))
            if inter_b:
                l1_group(0, tt, *slabs[0])
        if not inter_b:
            for tt in range(TT):
                l1_group(0, tt, *slabs[0])

        # ===== C: remaining L1 slabs ===================================
        # In the last slab, the per-token L2 scale finalize is emitted
        # right behind each group so it overlaps the L1 tail.
        for e in range(1, NE1):
            if e + 1 < NE1:
                slabs[e + 1] = load_slab1(e + 1)
            for tt in range(TT):
                l1_group(e, tt, *slabs[e])
                if e == NE1 - 1:
                    finalize(stv2, m2s, INNER, r2s, c2s, slice(tt, tt + 1))
                    nc.vector.tensor_scalar(beta2s[:, tt:tt + 1],
                                            c2s[:, tt:tt + 1],
                                            mws[:, 1:2], None, op0=ALU.mult)
                    col_to_row(beta2s[:, tt:tt + 1],
                               beta2row[0:1, tt * P:(tt + 1) * P])
            del slabs[e - 1]

        nc.gpsimd.partition_broadcast(bb0, beta2row[0:1, 0:512])
        nc.gpsimd.partition_broadcast(bb1, beta2row[0:1, 512:1024])
        bbs = [bb0, bb1]

        q2T = qTp.tile([P, KI, TOK], BF16, tag="qT", name="q2T")

        def q2_build(tt):
            for icp in range(INNER // 1024):
                hr = hrp.tile([P, 1024], BF16, tag="hr")
                nc.sync.dma_start(
                    hr, h_dram[tt, :, icp * 1024:(icp + 1) * 1024])
                q2c = q2cp.tile([P, 1024], BF16, tag="q2c")
                for hh in range(2):
                    hq = hqp.tile([P, 512], F32, tag="hq")
                    nc.scalar.activation(hq, hr[:, hh * 512:(hh + 1) * 512],
                                         AF.Copy, bias=C_RND,
                                         scale=r2s[:, tt:tt + 1])
                    nc.vector.tensor_scalar(q2c[:, hh * 512:(hh + 1) * 512],
                                            hq, C_RND, None,
                                            op0=ALU.subtract)
                pst = ps_t.tile([P, 1024], BF16, tag="pst")
                for j in range(8):
                    pe(nc.tensor.transpose(
                        pst[:, j * P:(j + 1) * P],
                        q2c[:, j * P:(j + 1) * P], identb))
                nc.vector.tensor_copy(
                    out=q2T[:, 8 * icp:8 * (icp + 1), tt * P:(tt + 1) * P],
                    in_=pst.rearrange("p (a b) -> p a b", b=P))

        # ===== E: L2 o-bands, token-half sweeps ========================
        # tg0's bands run while tg1's q2T is still being rebuilt (the
        # build transposes slot between band matmul groups on the PE
        # queue); w2 slabs are streamed once per sweep.
        def band(b, s, tg):
            w2h = [wring.tile([P, KI // 2, BO], BF16, tag="w",
                              name=f"ws2_{s}_{b}_{h}") for h in range(2)]
            for h in range(2):
                nc.sync.dma_start(
                    w2h[h], w2s_d[b * P:(b + 1) * P,
                                  h * (KI // 2) * BO:
                                  (h + 1) * (KI // 2) * BO].rearrange(
                        "p (k o) -> p k o", o=BO))
            pg2 = ps_g.tile([P, 512], F32, tag="psg", name=f"pb{s}_{b}")
            tsl = slice(tg * 512, (tg + 1) * 512)
            for kc in range(KI):
                h, k = divmod(kc, KI // 2)
                pe(nc.tensor.matmul(pg2, w2h[h][:, k, :],
                                    q2T[:, kc, tsl],
                                    start=(kc == 0),
                                    stop=(kc == KI - 1)))
            nc.vector.tensor_tensor(pg2, pg2, bbs[tg], op=ALU.mult)
            ob = outp.tile([P, 512], BF16, tag="ob")
            nc.vector.tensor_scalar(ob, pg2, b2c[:, b:b + 1], None,
                                    op0=ALU.add)
            nc.sync.dma_start(out_d[b * BO:(b + 1) * BO, tsl], ob)

        def band2(b):
            w2h = [wring.tile([P, KI // 2, BO], BF16, tag="w",
                              name=f"w2s_{b}_{h}") for h in range(2)]
            for h in range(2):
                nc.sync.dma_start(
                    w2h[h], w2s_d[b * P:(b + 1) * P,
                                  h * (KI // 2) * BO:
                                  (h + 1) * (KI // 2) * BO].rearrange(
                        "p (k o) -> p k o", o=BO))
            for tg in range(2):
                pg2 = ps_g.tile([P, 512], F32, tag="psg",
                                name=f"pb2_{b}_{tg}")
                tsl = slice(tg * 512, (tg + 1) * 512)
                for kc in range(KI):
                    h, k = divmod(kc, KI // 2)
                    pe(nc.tensor.matmul(pg2, w2h[h][:, k, :],
                                        q2T[:, kc, tsl],
                                        start=(kc == 0),
                                        stop=(kc == KI - 1)))
                nc.vector.tensor_tensor(pg2, pg2, bbs[tg], op=ALU.mult)
                ob = outp.tile([P, 512], BF16, tag="ob")
                nc.vector.tensor_scalar(ob, pg2, b2c[:, b:b + 1], None,
                                        op0=ALU.add)
                nc.sync.dma_start(out_d[b * BO:(b + 1) * BO, tsl], ob)

        if weave:
            for tt in range(4):
                q2_build(tt)
            for b in range(NB2):
                band(b, 0, 0)
                if b < 4:
                    q2_build(4 + b)
            for b in range(NB2):
                band(b, 1, 1)
        else:
            for tt in range(TT):
                q2_build(tt)
            for b in range(NB2):
                band2(b)

    nc.compile()
    return nc


@functools.lru_cache(maxsize=4)
def _get_nc(fp8_l1=True):
    import os
    inter_b = os.environ.get("K_INTER_B", "1") == "1"
    weave = os.environ.get("K_WEAVE", "0") == "1"
    return build(fp8_l1, inter_b, weave)


def _prep_weights(w1, w2, fp8_l1):
    w1 = np.asarray(w1, dtype=np.float32)
    w2 = np.asarray(w2, dtype=np.float32)
    mw1 = float(np.mean(np.abs(w1), dtype=np.float64))
    mw2 = float(np.mean(np.abs(w2), dtype=np.float64))
    wdt1 = ml_dtypes.float8_e4m3 if fp8_l1 else ml_dtypes.bfloat16
    s1t = np.sign(w1).T.astype(wdt1)        # [DIM, INNER]
    s2t = np.sign(w2).T.astype(ml_dtypes.bfloat16)  # [INNER, OUT]
    # w1s[e, p, dc, c] = s1t[dc*128+p, e*512+c]
    w1s = np.ascontiguousarray(
        s1t.reshape(KD, P, NE1, SL1).transpose(2, 1, 0, 3)
        .reshape(NE1 * P, KD * SL1))
    # w2s[b, p, kc, o] = s2t[kc*128+p, b*128+o]
    w2s = np.ascontiguousarray(
        s2t.reshape(KI, P, NB2, BO).transpose(2, 1, 0, 3)
        .reshape(NB2 * P, KI * BO))
    return w1s, w2s, mw1, mw2


def kernel(x, w1, b1, w2, b2, _trace=False, _fp8=True):
    nc = _get_nc(_fp8)
    xf = np.ascontiguousarray(x.reshape(B * S, DIM), dtype=np.float32)
    w1s, w2s, mw1, mw2 = _prep_weights(w1, w2, _fp8)
    b2f = np.asarray(b2, dtype=np.float32)
    common = {
        "w1s": w1s,
        "w2s": w2s,
        "b1": np.asarray(b1, dtype=np.float32).reshape(1, INNER).astype(
            ml_dtypes.bfloat16),
        "b2c": np.ascontiguousarray(
            b2f.reshape(NB2, P).T.astype(np.float32)),
        "mws": np.tile(np.array([[mw1 / 127.0, mw2 / 127.0]],
                                dtype=np.float32), (P, 1)),
        "identf": np.eye(P, dtype=np.float32),
        "identb": np.eye(P, dtype=np.float32).astype(ml_dtypes.bfloat16),
    }
    in_maps = []
    for c in range(NCORES):
        in_maps.append({
            "x": xf[c * TOK:(c + 1) * TOK],
            **common,
        })
    res = run_bass_kernel_spmd(nc, in_maps, core_ids=list(range(NCORES)),
                               trace=_trace)
    out = np.concatenate(
        [res.results[c]["out"].astype(np.float32).T for c in range(NCORES)],
        axis=0)
    out = out.reshape(B, S, DIM)
    if _trace:
        return out, res
    return out


# revision 5
# speedup vs baseline: 1.0764x; 1.0057x over previous
"""BitFeedForward (BitNet-style FFN) Trainium2 kernel — 8-core data parallel.

kernel(**inputs) takes the FULL unsharded inputs of
nn_BitFeedForward_25280177504455:
    x  [4, 2048, 2048] f32, w1 [8192, 2048], b1 [8192],
    w2 [2048, 8192], b2 [2048]
and returns the full [4, 2048, 2048] f32 output.

Sharding: data-parallel over tokens (1024 tokens/core).  The host
precomputes the ternary weight form sign(w) (exact, a pure dtype/layout
transform of the input) plus the exact scalar mean|w| for each matrix,
and ships sign(w1) as fp8_e4m3 / sign(w2) as bf16 in slab-contiguous
layouts (16KB contiguous per partition line -> minimal DMA descriptor
count).  No on-device sign conversion and no sampled-mean pass.

Layer 1 runs the matmul in fp8 DoubleRow perf mode (contraction 256 per
instruction): activations are quantized per-token to int8 values and
then rounded to the fp8e4 grid (the only lossy step, ~1.8e-2 final max
rel err vs the fp32 reference, verified against the reference pipeline
in numpy).  Layer 2 activations stay exact bf16 (fp8 there would break
the 2e-2 budget).

Per-core flow:
  B. per token-tile: x stats -> per-tile scale finalize -> quantize
     (C_RND round trick) -> PE-transpose -> q1T (fp8) resident in SBUF.
  C. L1: stream w1 fp8 slabs, 8 DoubleRow matmuls per psum group,
     bias via vector ops on PSUM, gelu on scalar engine -> h bf16
     staged and spilled to DRAM in 2KB/partition chunks; per-token
     bn_stats/absmax accumulate.
  D. batched L2 scale finalize; q2 rebuilt from h (bf16) and
     PE-transposed into q2T (SBUF buffer aliased over q1T's).
  E. L2: stream w2 bf16 slab halves, 64 matmuls per psum group
     [out 128 x tok 512], beta2 scale via vector, bias b2 via scalar
     activation, out written bf16 [o, t]; host transposes and upcasts.
"""
import functools

import numpy as np
import ml_dtypes

from contextlib import ExitStack

import concourse.bacc as bacc
import concourse.tile as tile
from concourse import mybir
from concourse.bass_utils import run_bass_kernel_spmd

F32 = mybir.dt.float32
BF16 = mybir.dt.bfloat16
FP8 = mybir.dt.float8e4

EPS_RMS = 1e-6
EPS_Q = 1e-5
# v + C lands in [2^23, 2^24) where fp32 spacing is 1.0 -> RNE integer round
C_RND = float(1.5 * 2.0**23)
P = 128
AX = mybir.AxisListType
ALU = mybir.AluOpType
AF = mybir.ActivationFunctionType
DR = mybir.MatmulPerfMode.DoubleRow

NCORES = 8
B, S, DIM = 4, 2048, 2048
INNER = 8192
OUT = DIM
TOK = B * S // NCORES   # 1024 tokens per core
TT = TOK // P           # 8 token tiles
KD = DIM // P           # 16 contraction chunks for L1
KI = INNER // P         # 64 contraction chunks for L2
NE1 = 16                # L1 slabs (512 inner cols each)
SL1 = INNER // NE1      # 512
NB2 = 16                # L2 o-bands (128 out cols each)
BO = OUT // NB2         # 128


def build(fp8_l1=True, inter_b=True, weave=True):
    from concourse.tile_rust import add_dep_helper

    nc = bacc.Bacc("TRN2", enable_partition_id=False, num_devices=NCORES)

    WDT1 = FP8 if fp8_l1 else BF16
    x_d = nc.dram_tensor("x", [TOK, DIM], F32, kind="ExternalInput")
    # host slab layouts: w1s [e, p, dc, c] flat [NE1*P, KD*SL1],
    # w2s [b, p, kc, o] flat [NB2*P, KI*BO]; each slab row is contiguous.
    w1s_d = nc.dram_tensor("w1s", [NE1 * P, KD * SL1], WDT1,
                           kind="ExternalInput")
    w2s_d = nc.dram_tensor("w2s", [NB2 * P, KI * BO], BF16,
                           kind="ExternalInput")
    b1_d = nc.dram_tensor("b1", [1, INNER], BF16, kind="ExternalInput")
    b2c_d = nc.dram_tensor("b2c", [P, NB2], F32, kind="ExternalInput")
    mws_d = nc.dram_tensor("mws", [P, 2], F32, kind="ExternalInput")
    idf_d = nc.dram_tensor("identf", [P, P], F32, kind="ExternalInput")
    idb_d = nc.dram_tensor("identb", [P, P], BF16, kind="ExternalInput")
    out_d = nc.dram_tensor("out", [OUT, TOK], BF16, kind="ExternalOutput")

    with ExitStack() as ctx:
        tc = ctx.enter_context(tile.TileContext(nc))
        pool = lambda name, bufs, space="SBUF": ctx.enter_context(
            tc.tile_pool(name=name, bufs=bufs, space=space))

        consts = pool("consts", 1)
        xp = pool("xp", 2)            # f32 x half-tiles [P, 1024]
        q1p = pool("q1p", 2)          # bf16 q1 staging [P, 1024]
        wring = pool("wring", 4)      # weight slab halves (8KB/partition)
        qTp = pool("qTp", 1)          # q1T then q2T (aliased buffer)
        hcp = pool("hcp", 3)          # h bf16 spill tiles [P, 512]
        hrp = pool("hrp", 2)          # h bf16 reload [P, 1024]
        hqp = pool("hqp", 2)          # f32 q2 round staging [P, 512]
        q2cp = pool("q2cp", 2)        # bf16 q2 [P, 1024]
        btep = pool("btep", 2)        # f32 bias_te [P, 512]
        bbp = pool("bbp", 2)          # b1bc slab broadcast
        bch = pool("bch", 1)          # b1 row slices
        outp = pool("outp", 2)        # bf16 output drains
        vecs = pool("vecs", 2)
        pers = pool("pers", 1)
        dram = pool("dram", 1, "DRAM")
        ps_g = pool("ps_g", 4, "PSUM")
        ps_t = pool("ps_t", 2, "PSUM")
        ps_v = pool("ps_v", 2, "PSUM")

        identf = consts.tile([P, P], F32)
        identb = consts.tile([P, P], BF16)
        b2c = consts.tile([P, NB2], F32)
        mws = consts.tile([P, 2], F32)
        nc.sync.dma_start(identf, idf_d[:, :])
        nc.sync.dma_start(identb, idb_d[:, :])
        nc.sync.dma_start(b2c, b2c_d[:, :])
        nc.sync.dma_start(mws, mws_d[:, :])

        h_dram = dram.tile([TT, P, INNER], BF16)

        state = {"pe": None}

        def pe(instr):
            if state["pe"] is not None:
                add_dep_helper(instr.ins, state["pe"].ins, sync=False,
                               reason="pe chain")
            state["pe"] = instr
            return instr

        # ---- persistent scalars/vectors ----
        stvx = pers.tile([P, TT, 4, 6], F32, tag="stvx")
        M1s = pers.tile([P, TT], F32, tag="M1s")
        r1s = pers.tile([P, TT], F32, tag="r1s")
        c1s = pers.tile([P, TT], F32, tag="c1s")
        beta1s = pers.tile([P, TT], F32, tag="beta1s")
        rb1s = pers.tile([P, TT], F32, tag="rb1s")
        r2s = pers.tile([P, TT], F32, tag="r2s")
        m2s = pers.tile([P, TT], F32, tag="m2s")
        c2s = pers.tile([P, TT], F32, tag="c2s")
        beta2s = pers.tile([P, TT], F32, tag="beta2s")
        beta2row = pers.tile([1, TOK], F32, tag="beta2row")
        stv2 = pers.tile([P, TT, NE1, 6], F32, tag="stv2")
        bb0 = pers.tile([P, 512], F32, tag="bb0")
        bb1 = pers.tile([P, 512], F32, tag="bb1")

        def quant_scale(M_all, r_all, sl):
            # r = 127/absmax: exact because absmax(x/rms/sqrt(W)) >=
            # 1/sqrt(W) >> EPS_Q, so the reference's clip never fires.
            nc.vector.reciprocal(r_all[:, sl], M_all[:, sl])
            nc.vector.tensor_scalar_mul(r_all[:, sl], r_all[:, sl], 127.0)

        def beta_fin(stv_all, M_all, WID, c_all, sl, mw_col, beta_out,
                     rb_out):
            # beta = max(absmax(xn), eps)*mean|w|/127; off the quant
            # critical path (consumed only at psum drain time).
            mvs = vecs.tile([P, TT, 2], F32, tag="bn_mvs")
            for tt in range(sl.start, sl.stop):
                nc.vector.bn_aggr(mvs[:, tt, :], stv_all[:, tt, :, :])
            msq = vecs.tile([P, TT], F32, tag="msqs")
            m_ = mvs[:, sl, 0]
            v_ = mvs[:, sl, 1]
            q_ = msq[:, sl]
            nc.vector.tensor_tensor(q_, m_, m_, op=ALU.mult)
            nc.vector.tensor_tensor(q_, q_, v_, op=ALU.add)
            nc.vector.tensor_scalar_add(q_, q_, EPS_RMS)
            y = vecs.tile([P, TT], F32, tag="sq_ys")
            nc.scalar.sqrt(y[:, sl], q_)
            d_ = vecs.tile([P, TT], F32, tag="sq_ds")
            nc.vector.reciprocal(d_[:, sl], y[:, sl])
            nc.vector.tensor_tensor(d_[:, sl], q_, d_[:, sl], op=ALU.mult)
            nc.vector.tensor_tensor(y[:, sl], y[:, sl], d_[:, sl],
                                    op=ALU.add)
            nc.vector.tensor_scalar_mul(y[:, sl], y[:, sl],
                                        0.5 * (float(WID) ** 0.5))
            a = vecs.tile([P, TT], F32, tag="as_")
            nc.vector.reciprocal(a[:, sl], y[:, sl])
            nc.vector.tensor_tensor(c_all[:, sl], a[:, sl], M_all[:, sl],
                                    op=ALU.mult)
            nc.vector.tensor_scalar_max(c_all[:, sl], c_all[:, sl], EPS_Q)
            nc.vector.tensor_scalar(beta_out[:, sl], c_all[:, sl],
                                    mw_col, None, op0=ALU.mult)
            if rb_out is not None:
                nc.vector.reciprocal(rb_out[:, sl], beta_out[:, sl])

        def col_to_row(col, row_slice):
            pst = ps_v.tile([1, P], F32, tag="psv")
            pe(nc.tensor.transpose(pst, col, identf))
            nc.scalar.copy(row_slice, pst)

        def load_slab1(e):
            wh = [wring.tile([P, KD // 2, SL1], WDT1, tag="w",
                             name=f"ws1_{e}_{h}") for h in range(2)]
            for h in range(2):
                nc.sync.dma_start(
                    wh[h], w1s_d[e * P:(e + 1) * P,
                                 h * (KD // 2) * SL1:
                                 (h + 1) * (KD // 2) * SL1].rearrange(
                        "p (k c) -> p k c", c=SL1))
            bc = bch.tile([1, SL1], BF16, tag="bc")
            nc.sync.dma_start(bc, b1_d[0:1, e * SL1:(e + 1) * SL1])
            b1bc = bbp.tile([P, SL1], BF16, tag="b1bc", name=f"b1bc_{e}")
            nc.gpsimd.partition_broadcast(b1bc, bc)
            return wh, b1bc

        def l1_group(e, tt, wh, b1bc):
            pg = ps_g.tile([P, SL1], F32, tag="psg")
            ts_ = slice(tt * P, (tt + 1) * P)
            if fp8_l1:
                for dcp in range(KD // 2):
                    h, k = divmod(dcp, 4)
                    pe(nc.tensor.matmul(
                        pg, q1T[:, 2 * dcp:2 * dcp + 2, ts_],
                        wh[h][:, 2 * k:2 * k + 2, :],
                        start=(dcp == 0), stop=(dcp == KD // 2 - 1),
                        perf_mode=DR))
            else:
                for dc in range(KD):
                    h, k = divmod(dc, KD // 2)
                    pe(nc.tensor.matmul(
                        pg, q1T[:, dc, ts_], wh[h][:, k, :],
                        start=(dc == 0), stop=(dc == KD - 1)))
            bias_te = btep.tile([P, SL1], BF16, tag="bte")
            nc.vector.tensor_scalar(bias_te, b1bc, rb1s[:, tt:tt + 1],
                                    None, op0=ALU.mult)
            nc.vector.tensor_tensor(pg, pg, bias_te, op=ALU.add)
            hc = hcp.tile([P, SL1], BF16, tag="hc")
            nc.scalar.activation(hc, pg, AF.Gelu,
                                 scale=beta1s[:, tt:tt + 1])
            nc.vector.bn_stats(stv2[:, tt, e, :], hc)
            mx = vecs.tile([P, 1], F32, tag="mx")
            nc.vector.tensor_reduce(mx, hc, axis=AX.X, op=ALU.max,
                                    apply_absolute_value=True)
            m2 = m2s[:, tt:tt + 1]
            if e == 0:
                nc.vector.tensor_copy(out=m2, in_=mx)
            else:
                nc.vector.tensor_tensor(m2, m2, mx, op=ALU.max)
            nc.sync.dma_start(h_dram[tt, :, e * 512:(e + 1) * 512], hc)

        # ===== B: per-tile x stats + finalize + quantize + transpose ===
        # L1 slab 0 (and 1) stream in during the stats pass; the first
        # matmul group for token tile tt is emitted right behind tt's
        # transposes so the PE queue never drains behind phase B.
        q1T = qTp.tile([P, KD, TOK], WDT1, tag="qT", name="q1T")
        slabs = {0: load_slab1(0), 1: load_slab1(1)}
        for tt in range(TT):
            xh = []
            for h in range(2):
                xt = xp.tile([P, 1024], F32, tag="x", name=f"x_{tt}_{h}")
                nc.sync.dma_start(
                    xt, x_d[tt * P:(tt + 1) * P, h * 1024:(h + 1) * 1024])
                xh.append(xt)
                for c in range(2):
                    nc.vector.bn_stats(stvx[:, tt, 2 * h + c, :],
                                       xt[:, c * 512:(c + 1) * 512])
                mx = vecs.tile([P, 1], F32, tag="mx")
                nc.vector.tensor_reduce(mx, xt, axis=AX.X, op=ALU.max,
                                        apply_absolute_value=True)
                M1 = M1s[:, tt:tt + 1]
                if h == 0:
                    nc.vector.tensor_copy(out=M1, in_=mx)
                else:
                    nc.vector.tensor_tensor(M1, M1, mx, op=ALU.max)
            quant_scale(M1s, r1s, slice(tt, tt + 1))
            beta_fin(stvx, M1s, DIM, c1s, slice(tt, tt + 1), mws[:, 0:1],
                     beta1s, rb1s)
            # v = x*r1 + C_RND ; q1 = v - C_RND (RNE integer round)
            for h in range(2):
                nc.vector.tensor_scalar(xh[h], xh[h], r1s[:, tt:tt + 1],
                                        C_RND, op0=ALU.mult, op1=ALU.add)
                q1 = q1p.tile([P, 1024], BF16, tag="q1")
                nc.vector.tensor_scalar(q1, xh[h], C_RND, None,
                                        op0=ALU.subtract)
                pst = ps_t.tile([P, 1024], BF16, tag="pst")
                for j in range(8):
                    pe(nc.tensor.transpose(
                        pst[:, j * P:(j + 1) * P],
                        q1[:, j * P:(j + 1) * P], identb))
                nc.vector.tensor_copy(
                    out=q1T[:, 8 * h:8 * (h + 1), tt * P:(tt + 1) * P],
                    in_=pst.rearrange("p (a b) -> p a b", b=P))
            if inter_b:
                l1_group(0, tt, *slabs[0])
        if not inter_b:
            for tt in range(TT):
                l1_group(0, tt, *slabs[0])

        # ===== C: remaining L1 slabs ===================================
        # In the last slab, the per-token L2 scale finalize is emitted
        # right behind each group so it overlaps the L1 tail.
        for e in range(1, NE1):
            if e + 1 < NE1:
                slabs[e + 1] = load_slab1(e + 1)
            for tt in range(TT):
                l1_group(e, tt, *slabs[e])
                if e == NE1 - 1:
                    quant_scale(m2s, r2s, slice(tt, tt + 1))
            del slabs[e - 1]

        beta_fin(stv2, m2s, INNER, c2s, slice(0, TT), mws[:, 1:2],
                 beta2s, None)
        for tt in range(TT):
            col_to_row(beta2s[:, tt:tt + 1],
                       beta2row[0:1, tt * P:(tt + 1) * P])
        nc.gpsimd.partition_broadcast(bb0, beta2row[0:1, 0:512])
        nc.gpsimd.partition_broadcast(bb1, beta2row[0:1, 512:1024])
        bbs = [bb0, bb1]

        q2T = qTp.tile([P, KI, TOK], BF16, tag="qT", name="q2T")

        def q2_build(tt):
            for icp in range(INNER // 1024):
                hr = hrp.tile([P, 1024], BF16, tag="hr")
                nc.sync.dma_start(
                    hr, h_dram[tt, :, icp * 1024:(icp + 1) * 1024])
                q2c = q2cp.tile([P, 1024], BF16, tag="q2c")
                for hh in range(2):
                    hq = hqp.tile([P, 512], F32, tag="hq")
                    nc.vector.tensor_scalar(hq, hr[:, hh * 512:(hh + 1) * 512],
                                            r2s[:, tt:tt + 1], C_RND,
                                            op0=ALU.mult, op1=ALU.add)
                    nc.vector.tensor_scalar(q2c[:, hh * 512:(hh + 1) * 512],
                                            hq, C_RND, None,
                                            op0=ALU.subtract)
                pst = ps_t.tile([P, 1024], BF16, tag="pst")
                for j in range(8):
                    pe(nc.tensor.transpose(
                        pst[:, j * P:(j + 1) * P],
                        q2c[:, j * P:(j + 1) * P], identb))
                nc.vector.tensor_copy(
                    out=q2T[:, 8 * icp:8 * (icp + 1), tt * P:(tt + 1) * P],
                    in_=pst.rearrange("p (a b) -> p a b", b=P))

        # ===== E: L2 o-bands, token-half sweeps ========================
        # tg0's bands run while tg1's q2T is still being rebuilt (the
        # build transposes slot between band matmul groups on the PE
        # queue); w2 slabs are streamed once per sweep.
        def band(b, s, tg):
            w2h = [wring.tile([P, KI // 2, BO], BF16, tag="w",
                              name=f"ws2_{s}_{b}_{h}") for h in range(2)]
            for h in range(2):
                nc.sync.dma_start(
                    w2h[h], w2s_d[b * P:(b + 1) * P,
                                  h * (KI // 2) * BO:
                                  (h + 1) * (KI // 2) * BO].rearrange(
                        "p (k o) -> p k o", o=BO))
            pg2 = ps_g.tile([P, 512], F32, tag="psg", name=f"pb{s}_{b}")
            tsl = slice(tg * 512, (tg + 1) * 512)
            for kc in range(KI):
                h, k = divmod(kc, KI // 2)
                pe(nc.tensor.matmul(pg2, w2h[h][:, k, :],
                                    q2T[:, kc, tsl],
                                    start=(kc == 0),
                                    stop=(kc == KI - 1)))
            nc.vector.tensor_tensor(pg2, pg2, bbs[tg], op=ALU.mult)
            ob = outp.tile([P, 512], BF16, tag="ob")
            nc.vector.tensor_scalar(ob, pg2, b2c[:, b:b + 1], None,
                                    op0=ALU.add)
            nc.sync.dma_start(out_d[b * BO:(b + 1) * BO, tsl], ob)

        def band2(b):
            w2h = [wring.tile([P, KI // 2, BO], BF16, tag="w",
                              name=f"w2s_{b}_{h}") for h in range(2)]
            for h in range(2):
                nc.sync.dma_start(
                    w2h[h], w2s_d[b * P:(b + 1) * P,
                                  h * (KI // 2) * BO:
                                  (h + 1) * (KI // 2) * BO].rearrange(
                        "p (k o) -> p k o", o=BO))
            for tg in range(2):
                pg2 = ps_g.tile([P, 512], F32, tag="psg",
                                name=f"pb2_{b}_{tg}")
                tsl = slice(tg * 512, (tg + 1) * 512)
                for kc in range(KI):
                    h, k = divmod(kc, KI // 2)
                    pe(nc.tensor.matmul(pg2, w2h[h][:, k, :],
                                        q2T[:, kc, tsl],
                                        start=(kc == 0),
                                        stop=(kc == KI - 1)))
                nc.vector.tensor_tensor(pg2, pg2, bbs[tg], op=ALU.mult)
                ob = outp.tile([P, 512], BF16, tag="ob")
                nc.vector.tensor_scalar(ob, pg2, b2c[:, b:b + 1], None,
                                        op0=ALU.add)
                nc.sync.dma_start(out_d[b * BO:(b + 1) * BO, tsl], ob)

        if weave:
            for tt in range(4):
                q2_build(tt)
            for b in range(NB2):
                band(b, 0, 0)
                if b < 4:
                    q2_build(4 + b)
            for b in range(NB2):
                band(b, 1, 1)
        else:
            for tt in range(TT):
                q2_build(tt)
            for b in range(NB2):
                band2(b)

    nc.compile()
    return nc


@functools.lru_cache(maxsize=4)
def _get_nc(fp8_l1=True):
    return build(fp8_l1, inter_b=True, weave=False)


def _prep_weights(w1, w2, fp8_l1):
    w1 = np.asarray(w1, dtype=np.float32)
    w2 = np.asarray(w2, dtype=np.float32)
    mw1 = float(np.mean(np.abs(w1), dtype=np.float64))
    mw2 = float(np.mean(np.abs(w2), dtype=np.float64))
    wdt1 = ml_dtypes.float8_e4m3 if fp8_l1 else ml_dtypes.bfloat16
    s1t = np.sign(w1).T.astype(wdt1)        # [DIM, INNER]
    s2t = np.sign(w2).T.astype(ml_dtypes.bfloat16)  # [INNER, OUT]
    # w1s[e, p, dc, c] = s1t[dc*128+p, e*512+c]
    w1s = np.ascontiguousarray(
        s1t.reshape(KD, P, NE1, SL1).transpose(2, 1, 0, 3)
        .reshape(NE1 * P, KD * SL1))
    # w2s[b, p, kc, o] = s2t[kc*128+p, b*128+o]
    w2s = np.ascontiguousarray(
        s2t.reshape(KI, P, NB2, BO).transpose(2, 1, 0, 3)
        .reshape(NB2 * P, KI * BO))
    return w1s, w2s, mw1, mw2


def kernel(x, w1, b1, w2, b2, _trace=False, _fp8=True):
    nc = _get_nc(_fp8)
    xf = np.ascontiguousarray(x.reshape(B * S, DIM), dtype=np.float32)
    w1s, w2s, mw1, mw2 = _prep_weights(w1, w2, _fp8)
    b2f = np.asarray(b2, dtype=np.float32)
    common = {
        "w1s": w1s,
        "w2s": w2s,
        "b1": np.asarray(b1, dtype=np.float32).reshape(1, INNER).astype(
            ml_dtypes.bfloat16),
        "b2c": np.ascontiguousarray(
            b2f.reshape(NB2, P).T.astype(np.float32)),
        "mws": np.tile(np.array([[mw1 / 127.0, mw2 / 127.0]],
                                dtype=np.float32), (P, 1)),
        "identf": np.eye(P, dtype=np.float32),
        "identb": np.eye(P, dtype=np.float32).astype(ml_dtypes.bfloat16),
    }
    in_maps = []
    for c in range(NCORES):
        in_maps.append({
            "x": xf[c * TOK:(c + 1) * TOK],
            **common,
        })
    res = run_bass_kernel_spmd(nc, in_maps, core_ids=list(range(NCORES)),
                               trace=_trace)
    out = np.concatenate(
        [res.results[c]["out"].astype(np.float32).T for c in range(NCORES)],
        axis=0)
    out = out.reshape(B, S, DIM)
    if _trace:
        return out, res
    return out


# revision 6
# speedup vs baseline: 1.0961x; 1.0183x over previous
"""BitFeedForward (BitNet-style FFN) Trainium2 kernel — 8-core data parallel.

kernel(**inputs) takes the FULL unsharded inputs of
nn_BitFeedForward_25280177504455:
    x  [4, 2048, 2048] f32, w1 [8192, 2048], b1 [8192],
    w2 [2048, 8192], b2 [2048]
and returns the full [4, 2048, 2048] f32 output.

Sharding: data-parallel over tokens (1024 tokens/core).  The host
precomputes the ternary weight form sign(w) (exact, a pure dtype/layout
transform of the input) plus the exact scalar mean|w| for each matrix,
and ships sign(w1) as fp8_e4m3 / sign(w2) as bf16 in slab-contiguous
layouts (16KB contiguous per partition line -> minimal DMA descriptor
count).  No on-device sign conversion and no sampled-mean pass.

Layer 1 runs the matmul in fp8 DoubleRow perf mode (contraction 256 per
instruction): activations are quantized per-token to int8 values and
then rounded to the fp8e4 grid (the only lossy step, ~1.8e-2 final max
rel err vs the fp32 reference, verified against the reference pipeline
in numpy).  Layer 2 activations stay exact bf16 (fp8 there would break
the 2e-2 budget).

Per-core flow:
  B. per token-tile: x stats -> per-tile scale finalize -> quantize
     (C_RND round trick) -> PE-transpose -> q1T (fp8) resident in SBUF.
  C. L1: stream w1 fp8 slabs, 8 DoubleRow matmuls per psum group,
     bias via vector ops on PSUM, gelu on scalar engine -> h bf16
     staged and spilled to DRAM in 2KB/partition chunks; per-token
     bn_stats/absmax accumulate.
  D. batched L2 scale finalize; q2 rebuilt from h (bf16) and
     PE-transposed into q2T (SBUF buffer aliased over q1T's).
  E. L2: stream w2 bf16 slab halves, 64 matmuls per psum group
     [out 128 x tok 512], beta2 scale via vector, bias b2 via scalar
     activation, out written bf16 [o, t]; host transposes and upcasts.
"""
import functools

import numpy as np
import ml_dtypes

from contextlib import ExitStack

import concourse.bacc as bacc
import concourse.tile as tile
from concourse import mybir
from concourse.bass_utils import run_bass_kernel_spmd

F32 = mybir.dt.float32
BF16 = mybir.dt.bfloat16
FP8 = mybir.dt.float8e4

EPS_RMS = 1e-6
EPS_Q = 1e-5
# v + C lands in [2^23, 2^24) where fp32 spacing is 1.0 -> RNE integer round
C_RND = float(1.5 * 2.0**23)
P = 128
AX = mybir.AxisListType
ALU = mybir.AluOpType
AF = mybir.ActivationFunctionType
DR = mybir.MatmulPerfMode.DoubleRow

NCORES = 8
B, S, DIM = 4, 2048, 2048
INNER = 8192
OUT = DIM
TOK = B * S // NCORES   # 1024 tokens per core
TT = TOK // P           # 8 token tiles
KD = DIM // P           # 16 contraction chunks for L1
KI = INNER // P         # 64 contraction chunks for L2
NE1 = 16                # L1 slabs (512 inner cols each)
SL1 = INNER // NE1      # 512
NB2 = 16                # L2 o-bands (128 out cols each)
BO = OUT // NB2         # 128


def build(fp8_l1=True, inter_b=True, weave=True):
    from concourse.tile_rust import add_dep_helper

    nc = bacc.Bacc("TRN2", enable_partition_id=False, num_devices=NCORES)

    WDT1 = FP8 if fp8_l1 else BF16
    x_d = nc.dram_tensor("x", [TOK, DIM], F32, kind="ExternalInput")
    # host slab layouts: w1s [e, p, dc, c] flat [NE1*P, KD*SL1],
    # w2s [b, p, kc, o] flat [NB2*P, KI*BO]; each slab row is contiguous.
    w1s_d = nc.dram_tensor("w1s", [NE1 * P, KD * SL1], WDT1,
                           kind="ExternalInput")
    w2s_d = nc.dram_tensor("w2s", [NB2 * P, KI * BO], BF16,
                           kind="ExternalInput")
    b1_d = nc.dram_tensor("b1", [1, INNER], BF16, kind="ExternalInput")
    b2c_d = nc.dram_tensor("b2c", [P, NB2], F32, kind="ExternalInput")
    mws_d = nc.dram_tensor("mws", [P, 2], F32, kind="ExternalInput")
    idf_d = nc.dram_tensor("identf", [P, P], F32, kind="ExternalInput")
    idb_d = nc.dram_tensor("identb", [P, P], BF16, kind="ExternalInput")
    out_d = nc.dram_tensor("out", [OUT, TOK], BF16, kind="ExternalOutput")

    with ExitStack() as ctx:
        tc = ctx.enter_context(tile.TileContext(nc))
        pool = lambda name, bufs, space="SBUF": ctx.enter_context(
            tc.tile_pool(name=name, bufs=bufs, space=space))

        consts = pool("consts", 1)
        xp = pool("xp", 2)            # f32 x tiles [P, 2048]
        q1p = pool("q1p", 2)          # bf16 q1 staging [P, 1024]
        wring = pool("wring", 3)      # weight slab halves (8KB/partition)
        qTp = pool("qTp", 1)          # q1T then q2T (aliased buffer)
        hcp = pool("hcp", 3)          # h bf16 spill tiles [P, 512]
        hrp = pool("hrp", 2)          # h bf16 reload [P, 1024]
        hqp = pool("hqp", 2)          # f32 q2 round staging [P, 512]
        q2cp = pool("q2cp", 2)        # bf16 q2 [P, 1024]
        btep = pool("btep", 2)        # f32 bias_te [P, 512]
        bbp = pool("bbp", 2)          # b1bc slab broadcast
        bch = pool("bch", 1)          # b1 row slices
        outp = pool("outp", 2)        # bf16 output drains
        vecs = pool("vecs", 2)
        pers = pool("pers", 1)
        dram = pool("dram", 1, "DRAM")
        ps_g = pool("ps_g", 4, "PSUM")
        ps_t = pool("ps_t", 2, "PSUM")
        ps_v = pool("ps_v", 2, "PSUM")

        identf = consts.tile([P, P], F32)
        identb = consts.tile([P, P], BF16)
        b2c = consts.tile([P, NB2], F32)
        mws = consts.tile([P, 2], F32)
        nc.sync.dma_start(identf, idf_d[:, :])
        nc.sync.dma_start(identb, idb_d[:, :])
        nc.sync.dma_start(b2c, b2c_d[:, :])
        nc.sync.dma_start(mws, mws_d[:, :])

        h_dram = dram.tile([TT, P, INNER], BF16)

        state = {"pe": None}

        def pe(instr):
            if state["pe"] is not None:
                add_dep_helper(instr.ins, state["pe"].ins, sync=False,
                               reason="pe chain")
            state["pe"] = instr
            return instr

        # ---- persistent scalars/vectors ----
        stvx = pers.tile([P, TT, 4, 6], F32, tag="stvx")
        M1s = pers.tile([P, TT], F32, tag="M1s")
        r1s = pers.tile([P, TT], F32, tag="r1s")
        c1s = pers.tile([P, TT], F32, tag="c1s")
        beta1s = pers.tile([P, TT], F32, tag="beta1s")
        rb1s = pers.tile([P, TT], F32, tag="rb1s")
        r2s = pers.tile([P, TT], F32, tag="r2s")
        m2s = pers.tile([P, TT], F32, tag="m2s")
        c2s = pers.tile([P, TT], F32, tag="c2s")
        beta2s = pers.tile([P, TT], F32, tag="beta2s")
        beta2row = pers.tile([1, TOK], F32, tag="beta2row")
        stv2 = pers.tile([P, TT, NE1, 6], F32, tag="stv2")
        bb0 = pers.tile([P, 512], F32, tag="bb0")
        bb1 = pers.tile([P, 512], F32, tag="bb1")

        def quant_scale(M_all, r_all, sl):
            # r = 127/absmax: exact because absmax(x/rms/sqrt(W)) >=
            # 1/sqrt(W) >> EPS_Q, so the reference's clip never fires.
            nc.vector.reciprocal(r_all[:, sl], M_all[:, sl])
            nc.vector.tensor_scalar_mul(r_all[:, sl], r_all[:, sl], 127.0)

        def beta_fin(stv_all, M_all, WID, c_all, sl, mw_col, beta_out,
                     rb_out):
            # beta = max(absmax(xn), eps)*mean|w|/127; off the quant
            # critical path (consumed only at psum drain time).
            mvs = vecs.tile([P, TT, 2], F32, tag="bn_mvs")
            for tt in range(sl.start, sl.stop):
                nc.vector.bn_aggr(mvs[:, tt, :], stv_all[:, tt, :, :])
            msq = vecs.tile([P, TT], F32, tag="msqs")
            m_ = mvs[:, sl, 0]
            v_ = mvs[:, sl, 1]
            q_ = msq[:, sl]
            nc.vector.tensor_tensor(q_, m_, m_, op=ALU.mult)
            nc.vector.tensor_tensor(q_, q_, v_, op=ALU.add)
            nc.vector.tensor_scalar_add(q_, q_, EPS_RMS)
            y = vecs.tile([P, TT], F32, tag="sq_ys")
            nc.scalar.sqrt(y[:, sl], q_)
            d_ = vecs.tile([P, TT], F32, tag="sq_ds")
            nc.vector.reciprocal(d_[:, sl], y[:, sl])
            nc.vector.tensor_tensor(d_[:, sl], q_, d_[:, sl], op=ALU.mult)
            nc.vector.tensor_tensor(y[:, sl], y[:, sl], d_[:, sl],
                                    op=ALU.add)
            nc.vector.tensor_scalar_mul(y[:, sl], y[:, sl],
                                        0.5 * (float(WID) ** 0.5))
            a = vecs.tile([P, TT], F32, tag="as_")
            nc.vector.reciprocal(a[:, sl], y[:, sl])
            nc.vector.tensor_tensor(c_all[:, sl], a[:, sl], M_all[:, sl],
                                    op=ALU.mult)
            nc.vector.tensor_scalar_max(c_all[:, sl], c_all[:, sl], EPS_Q)
            nc.vector.tensor_scalar(beta_out[:, sl], c_all[:, sl],
                                    mw_col, None, op0=ALU.mult)
            if rb_out is not None:
                nc.vector.reciprocal(rb_out[:, sl], beta_out[:, sl])

        def col_to_row(col, row_slice):
            pst = ps_v.tile([1, P], F32, tag="psv")
            pe(nc.tensor.transpose(pst, col, identf))
            nc.scalar.copy(row_slice, pst)

        def load_slab1(e):
            wh = [wring.tile([P, KD // 2, SL1], WDT1, tag="w",
                             name=f"ws1_{e}_{h}") for h in range(2)]
            for h in range(2):
                nc.sync.dma_start(
                    wh[h], w1s_d[e * P:(e + 1) * P,
                                 h * (KD // 2) * SL1:
                                 (h + 1) * (KD // 2) * SL1].rearrange(
                        "p (k c) -> p k c", c=SL1))
            bc = bch.tile([1, SL1], BF16, tag="bc")
            nc.sync.dma_start(bc, b1_d[0:1, e * SL1:(e + 1) * SL1])
            b1bc = bbp.tile([P, SL1], BF16, tag="b1bc", name=f"b1bc_{e}")
            nc.gpsimd.partition_broadcast(b1bc, bc)
            return wh, b1bc

        def l1_group(e, tt, wh, b1bc):
            pg = ps_g.tile([P, SL1], F32, tag="psg")
            ts_ = slice(tt * P, (tt + 1) * P)
            if fp8_l1:
                for dcp in range(KD // 2):
                    h, k = divmod(dcp, 4)
                    pe(nc.tensor.matmul(
                        pg, q1T[:, 2 * dcp:2 * dcp + 2, ts_],
                        wh[h][:, 2 * k:2 * k + 2, :],
                        start=(dcp == 0), stop=(dcp == KD // 2 - 1),
                        perf_mode=DR))
            else:
                for dc in range(KD):
                    h, k = divmod(dc, KD // 2)
                    pe(nc.tensor.matmul(
                        pg, q1T[:, dc, ts_], wh[h][:, k, :],
                        start=(dc == 0), stop=(dc == KD - 1)))
            bias_te = btep.tile([P, SL1], BF16, tag="bte")
            nc.vector.tensor_scalar(bias_te, b1bc, rb1s[:, tt:tt + 1],
                                    None, op0=ALU.mult)
            nc.vector.tensor_tensor(pg, pg, bias_te, op=ALU.add)
            hc = hcp.tile([P, SL1], BF16, tag="hc")
            nc.scalar.activation(hc, pg, AF.Gelu,
                                 scale=beta1s[:, tt:tt + 1])
            nc.vector.bn_stats(stv2[:, tt, e, :], hc)
            mx = vecs.tile([P, 1], F32, tag="mx")
            nc.vector.tensor_reduce(mx, hc, axis=AX.X, op=ALU.max,
                                    apply_absolute_value=True)
            m2 = m2s[:, tt:tt + 1]
            if e == 0:
                nc.vector.tensor_copy(out=m2, in_=mx)
            else:
                nc.vector.tensor_tensor(m2, m2, mx, op=ALU.max)
            nc.sync.dma_start(h_dram[tt, :, e * 512:(e + 1) * 512], hc)

        # ===== B: per-tile x stats + finalize + quantize + transpose ===
        # L1 slab 0 (and 1) stream in during the stats pass; the first
        # matmul group for token tile tt is emitted right behind tt's
        # transposes so the PE queue never drains behind phase B.
        q1T = qTp.tile([P, KD, TOK], WDT1, tag="qT", name="q1T")
        xts = {0: xp.tile([P, DIM], F32, tag="x", name="x_0")}
        nc.sync.dma_start(xts[0], x_d[0:P, :])
        slabs = {0: load_slab1(0), 1: load_slab1(1)}
        for tt in range(TT):
            if tt not in xts:
                xts[tt] = xp.tile([P, DIM], F32, tag="x", name=f"x_{tt}")
                nc.sync.dma_start(xts[tt],
                                  x_d[tt * P:(tt + 1) * P, :])
            xt = xts[tt]
            for c in range(4):
                nc.vector.bn_stats(stvx[:, tt, c, :],
                                   xt[:, c * 512:(c + 1) * 512])
            nc.vector.tensor_reduce(M1s[:, tt:tt + 1], xt, axis=AX.X,
                                    op=ALU.max, apply_absolute_value=True)
            quant_scale(M1s, r1s, slice(tt, tt + 1))
            beta_fin(stvx, M1s, DIM, c1s, slice(tt, tt + 1), mws[:, 0:1],
                     beta1s, rb1s)
            # v = x*r1 + C_RND ; q1 = v - C_RND (RNE integer round)
            for h in range(2):
                xsl = xt[:, h * 1024:(h + 1) * 1024]
                nc.vector.tensor_scalar(xsl, xsl, r1s[:, tt:tt + 1],
                                        C_RND, op0=ALU.mult, op1=ALU.add)
                q1 = q1p.tile([P, 1024], BF16, tag="q1")
                nc.vector.tensor_scalar(q1, xsl, C_RND, None,
                                        op0=ALU.subtract)
                pst = ps_t.tile([P, 1024], BF16, tag="pst")
                for j in range(8):
                    pe(nc.tensor.transpose(
                        pst[:, j * P:(j + 1) * P],
                        q1[:, j * P:(j + 1) * P], identb))
                nc.vector.tensor_copy(
                    out=q1T[:, 8 * h:8 * (h + 1), tt * P:(tt + 1) * P],
                    in_=pst.rearrange("p (a b) -> p a b", b=P))
            if inter_b:
                l1_group(0, tt, *slabs[0])
        if not inter_b:
            for tt in range(TT):
                l1_group(0, tt, *slabs[0])

        # ===== C: remaining L1 slabs ===================================
        # In the last slab, the per-token L2 scale finalize is emitted
        # right behind each group so it overlaps the L1 tail.
        for e in range(1, NE1):
            if e + 1 < NE1:
                slabs[e + 1] = load_slab1(e + 1)
            for tt in range(TT):
                l1_group(e, tt, *slabs[e])
                if e == NE1 - 1:
                    quant_scale(m2s, r2s, slice(tt, tt + 1))
            del slabs[e - 1]

        beta_fin(stv2, m2s, INNER, c2s, slice(0, TT), mws[:, 1:2],
                 beta2s, None)
        for tt in range(TT):
            col_to_row(beta2s[:, tt:tt + 1],
                       beta2row[0:1, tt * P:(tt + 1) * P])
        nc.gpsimd.partition_broadcast(bb0, beta2row[0:1, 0:512])
        nc.gpsimd.partition_broadcast(bb1, beta2row[0:1, 512:1024])
        bbs = [bb0, bb1]

        q2T = qTp.tile([P, KI, TOK], BF16, tag="qT", name="q2T")

        def q2_build(tt):
            for icp in range(INNER // 1024):
                hr = hrp.tile([P, 1024], BF16, tag="hr")
                nc.sync.dma_start(
                    hr, h_dram[tt, :, icp * 1024:(icp + 1) * 1024])
                q2c = q2cp.tile([P, 1024], BF16, tag="q2c")
                for hh in range(2):
                    hq = hqp.tile([P, 512], F32, tag="hq")
                    nc.vector.tensor_scalar(hq, hr[:, hh * 512:(hh + 1) * 512],
                                            r2s[:, tt:tt + 1], C_RND,
                                            op0=ALU.mult, op1=ALU.add)
                    nc.vector.tensor_scalar(q2c[:, hh * 512:(hh + 1) * 512],
                                            hq, C_RND, None,
                                            op0=ALU.subtract)
                pst = ps_t.tile([P, 1024], BF16, tag="pst")
                for j in range(8):
                    pe(nc.tensor.transpose(
                        pst[:, j * P:(j + 1) * P],
                        q2c[:, j * P:(j + 1) * P], identb))
                nc.vector.tensor_copy(
                    out=q2T[:, 8 * icp:8 * (icp + 1), tt * P:(tt + 1) * P],
                    in_=pst.rearrange("p (a b) -> p a b", b=P))

        # ===== E: L2 o-bands, token-half sweeps ========================
        # tg0's bands run while tg1's q2T is still being rebuilt (the
        # build transposes slot between band matmul groups on the PE
        # queue); w2 slabs are streamed once per sweep.
        def band(b, s, tg):
            w2h = [wring.tile([P, KI // 2, BO], BF16, tag="w",
                              name=f"ws2_{s}_{b}_{h}") for h in range(2)]
            for h in range(2):
                nc.sync.dma_start(
                    w2h[h], w2s_d[b * P:(b + 1) * P,
                                  h * (KI // 2) * BO:
                                  (h + 1) * (KI // 2) * BO].rearrange(
                        "p (k o) -> p k o", o=BO))
            pg2 = ps_g.tile([P, 512], F32, tag="psg", name=f"pb{s}_{b}")
            tsl = slice(tg * 512, (tg + 1) * 512)
            for kc in range(KI):
                h, k = divmod(kc, KI // 2)
                pe(nc.tensor.matmul(pg2, w2h[h][:, k, :],
                                    q2T[:, kc, tsl],
                                    start=(kc == 0),
                                    stop=(kc == KI - 1)))
            nc.vector.tensor_tensor(pg2, pg2, bbs[tg], op=ALU.mult)
            ob = outp.tile([P, 512], BF16, tag="ob")
            nc.vector.tensor_scalar(ob, pg2, b2c[:, b:b + 1], None,
                                    op0=ALU.add)
            nc.sync.dma_start(out_d[b * BO:(b + 1) * BO, tsl], ob)

        def band2(b):
            w2h = [wring.tile([P, KI // 2, BO], BF16, tag="w",
                              name=f"w2s_{b}_{h}") for h in range(2)]
            for h in range(2):
                nc.sync.dma_start(
                    w2h[h], w2s_d[b * P:(b + 1) * P,
                                  h * (KI // 2) * BO:
                                  (h + 1) * (KI // 2) * BO].rearrange(
                        "p (k o) -> p k o", o=BO))
            for tg in range(2):
                pg2 = ps_g.tile([P, 512], F32, tag="psg",
                                name=f"pb2_{b}_{tg}")
                tsl = slice(tg * 512, (tg + 1) * 512)
                for kc in range(KI):
                    h, k = divmod(kc, KI // 2)
                    pe(nc.tensor.matmul(pg2, w2h[h][:, k, :],
                                        q2T[:, kc, tsl],
                                        start=(kc == 0),
                                        stop=(kc == KI - 1)))
                nc.vector.tensor_tensor(pg2, pg2, bbs[tg], op=ALU.mult)
                ob = outp.tile([P, 512], BF16, tag="ob")
                nc.vector.tensor_scalar(ob, pg2, b2c[:, b:b + 1], None,
                                        op0=ALU.add)
                nc.sync.dma_start(out_d[b * BO:(b + 1) * BO, tsl], ob)

        if weave:
            for tt in range(4):
                q2_build(tt)
            for b in range(NB2):
                band(b, 0, 0)
                if b < 4:
                    q2_build(4 + b)
            for b in range(NB2):
                band(b, 1, 1)
        else:
            for tt in range(TT):
                q2_build(tt)
            for b in range(NB2):
                band2(b)

    nc.compile()
    return nc


@functools.lru_cache(maxsize=4)
def _get_nc(fp8_l1=True):
    return build(fp8_l1, inter_b=True, weave=False)


def _prep_weights(w1, w2, fp8_l1):
    w1 = np.asarray(w1, dtype=np.float32)
    w2 = np.asarray(w2, dtype=np.float32)
    mw1 = float(np.mean(np.abs(w1), dtype=np.float64))
    mw2 = float(np.mean(np.abs(w2), dtype=np.float64))
    wdt1 = ml_dtypes.float8_e4m3 if fp8_l1 else ml_dtypes.bfloat16
    s1t = np.sign(w1).T.astype(wdt1)        # [DIM, INNER]
    s2t = np.sign(w2).T.astype(ml_dtypes.bfloat16)  # [INNER, OUT]
    # w1s[e, p, dc, c] = s1t[dc*128+p, e*512+c]
    w1s = np.ascontiguousarray(
        s1t.reshape(KD, P, NE1, SL1).transpose(2, 1, 0, 3)
        .reshape(NE1 * P, KD * SL1))
    # w2s[b, p, kc, o] = s2t[kc*128+p, b*128+o]
    w2s = np.ascontiguousarray(
        s2t.reshape(KI, P, NB2, BO).transpose(2, 1, 0, 3)
        .reshape(NB2 * P, KI * BO))
    return w1s, w2s, mw1, mw2


def kernel(x, w1, b1, w2, b2, _trace=False, _fp8=True):
    nc = _get_nc(_fp8)
    xf = np.ascontiguousarray(x.reshape(B * S, DIM), dtype=np.float32)
    w1s, w2s, mw1, mw2 = _prep_weights(w1, w2, _fp8)
    b2f = np.asarray(b2, dtype=np.float32)
    common = {
        "w1s": w1s,
        "w2s": w2s,
        "b1": np.asarray(b1, dtype=np.float32).reshape(1, INNER).astype(
            ml_dtypes.bfloat16),
        "b2c": np.ascontiguousarray(
            b2f.reshape(NB2, P).T.astype(np.float32)),
        "mws": np.tile(np.array([[mw1 / 127.0, mw2 / 127.0]],
                                dtype=np.float32), (P, 1)),
        "identf": np.eye(P, dtype=np.float32),
        "identb": np.eye(P, dtype=np.float32).astype(ml_dtypes.bfloat16),
    }
    in_maps = []
    for c in range(NCORES):
        in_maps.append({
            "x": xf[c * TOK:(c + 1) * TOK],
            **common,
        })
    res = run_bass_kernel_spmd(nc, in_maps, core_ids=list(range(NCORES)),
                               trace=_trace)
    out = np.concatenate(
        [res.results[c]["out"].astype(np.float32).T for c in range(NCORES)],
        axis=0)
    out = out.reshape(B, S, DIM)
    if _trace:
        return out, res
    return out
